# revision 24
# baseline (speedup 1.0000x reference)
"""Longformer (sliding-window attention) forward pass on 8 Trainium2 NeuronCores.

Sharding: sequence-parallel. 8 shards of 1024 tokens (4 shards per batch
element). Each core keeps a 32-token halo on each side of its shard; the halo
is refreshed after every layer with a boundary-block AllGather collective +
an indirect-DMA neighbor pick (per-core offsets are input data, so the SPMD
program stays identical across cores).

Device layout: activations are "d-major" ([d on partitions, token on free]) so
every matmul contracts over the partition dimension without transposes. V is
produced token-major straight from its projection matmul because probs@V
contracts over keys. Attention scores are computed key-major [g, t]; softmax
uses exp(x) without max-subtraction (scores are bounded: layernormed inputs,
~N(0, 0.02^2) weights), with MULTIPLICATIVE 0/1 masking applied post-exp on
the vector engine, keeping mask work off the PE.

PE scheduling: the Tile list-scheduler pops ready instructions in emission
order, so independent big-GEMM streams (QKV / Wo / FFN) are given dedicated
PSUM banks (s0/s1) disjoint from attention's banks (sc0/sc1/pv0/pv1/seb) so
they weave into attention's dependency stalls, keeping the PE HAM clock warm
(idle PE re-throttles 2.4->1.2 GHz after ~3.4us). FFN W2 runs ko-major over
resident gelu tiles (2 cycling accumulator banks instead of 6 held ones);
next-layer weights are prefetched during FFN; next-layer Q/V chunks that
depend only on ln2-chunk0 are emitted first so the ln2-chunk1 scalar chain
and the halo collective hide behind PE work. Layernorm stats pack into one
PSUM bank, the scalar chain reads PSUM directly, and the row broadcast runs
as a GpSimd partition_broadcast so LN never blocks the PE stream banks; the
softmax denominator uses one batched reciprocal_approx_fast per query chunk.

Precision: bf16 matmul inputs / fp32 PSUM accumulation; residual stream bf16;
layernorm statistics fp32 (ones-matmuls); rsqrt via exp(-0.5*ln(var+eps)) on
the scalar engine.
"""

import os
import numpy as np
import ml_dtypes

import concourse.bass as bass
import concourse.bacc as bacc
import concourse.mybir as mybir
from concourse.tile import TileContext
from concourse.bass import IndirectOffsetOnAxis
from concourse.bass_utils import run_bass_kernel_spmd

FP32 = mybir.dt.float32
BF16 = mybir.dt.bfloat16
INT32 = mybir.dt.int32
ALU = mybir.AluOpType
AF = mybir.ActivationFunctionType
AX = mybir.AxisListType

# model dims
B, S, D, H, L_FULL, V, NCOUT = 2, 4096, 768, 12, 12, 50257, 16
DH = D // H            # 64
DFF = 4 * D            # 3072
W = 32                 # one-sided window
EPS = 1e-12
NC_CORES = 8
SHARDS_PER_B = 4
OWN = S // SHARDS_PER_B      # 1024 tokens per shard
EXT = OWN + 2 * W            # 1088 with halo
EXTP = 1152                  # EXT padded to 9*128 for the embedding gather
DK = D // 128                # 6 partition chunks of d
DFFK = DFF // 128            # 24 chunks of dff
NQC = OWN // 128             # 8 query chunks per shard
GW = 192                     # keys per 128-query chunk (128 + 2W + 64)

L = int(os.environ.get("KERNEL_LAYERS", str(L_FULL)))

TC_OWN = [(0, 512), (512, 512)]
TC_EXT = [(512, 512), (0, 512), (1024, EXT - 1024)]


def build_nc(n_layers: int):
    nc = bacc.Bacc("TRN2", target_bir_lowering=False, debug=False,
                   num_devices=NC_CORES)

    # ---------------- DRAM I/O ----------------
    emb_d = nc.dram_tensor("emb", [V, D], BF16, kind="ExternalInput")
    xids_d = nc.dram_tensor("xids", [EXTP // 128, 128], INT32, kind="ExternalInput")
    pos_d = nc.dram_tensor("postok", [EXTP, D], BF16, kind="ExternalInput")
    maskP_d = nc.dram_tensor("maskP", [3, 128, 512], BF16, kind="ExternalInput")
    hofs_d = nc.dram_tensor("hofs", [2 * DK, 128], INT32, kind="ExternalInput")
    ident_d = nc.dram_tensor("ident", [128, 128], BF16, kind="ExternalInput")
    wq_d = nc.dram_tensor("wq", [n_layers, D, D], BF16, kind="ExternalInput")
    wk_d = nc.dram_tensor("wk", [n_layers, D, D], BF16, kind="ExternalInput")
    wv_d = nc.dram_tensor("wv", [n_layers, D, D], BF16, kind="ExternalInput")
    wo_d = nc.dram_tensor("wo", [n_layers, D, D], BF16, kind="ExternalInput")
    w1_d = nc.dram_tensor("w1", [n_layers, D, DFF], BF16, kind="ExternalInput")
    w2_d = nc.dram_tensor("w2", [n_layers, DFF, D], BF16, kind="ExternalInput")
    fcw_d = nc.dram_tensor("fcw", [D, NCOUT], FP32, kind="ExternalInput")
    out_d = nc.dram_tensor("out", [NCOUT, 1], FP32, kind="ExternalOutput")

    # per-layer collective bounce buffers (internal DRAM)
    ag_in = [nc.dram_tensor(f"ag_in_{l}", [D, 2, W], BF16)
             for l in range(n_layers - 1)]
    ag_out = [nc.dram_tensor(f"ag_out_{l}", [NC_CORES, D, 2, W], BF16,
                             addr_space="Shared")
              for l in range(n_layers - 1)]

    wview = {}
    for name, t in (("wq", wq_d), ("wk", wk_d), ("wv", wv_d),
                    ("wo", wo_d), ("w1", w1_d), ("w2", w2_d)):
        wview[name] = t.ap().rearrange("l (a p) n -> l p a n", p=128)

    with TileContext(nc) as tc:
        with (
            tc.tile_pool(name="const", bufs=1) as cpool,
            tc.tile_pool(name="hpool", bufs=2) as hpool,
            tc.tile_pool(name="big", bufs=1) as bpool,
            tc.tile_pool(name="stream", bufs=3) as spool,
            tc.tile_pool(name="small", bufs=3) as smpool,
            tc.tile_pool(name="psum", bufs=1, space="PSUM") as ppool,
        ):
            pools = (hpool, bpool, spool, smpool, ppool)
            # ---------------- constants ----------------
            ones_col = cpool.tile([128, 1], BF16, tag="ones_col")
            nc.vector.memset(ones_col[:], 1.0)
            ones_row = cpool.tile([1, 128], BF16, tag="ones_row")
            nc.vector.memset(ones_row[:], 1.0)
            cneg_row = cpool.tile([1, 128], BF16, tag="cneg_row")
            nc.vector.memset(cneg_row[:], -1.0 / D)
            eps_col = cpool.tile([128, 1], FP32, tag="eps_col")
            nc.vector.memset(eps_col[:], EPS)
            ident = cpool.tile([128, 128], BF16, tag="ident")
            nc.sync.dma_start(ident[:], ident_d[:, :])
            ones_rows = cpool.tile([96, 128], BF16, tag="ones_rows")
            nc.vector.memset(ones_rows[:], 1.0)

            offs = cpool.tile([128, EXTP // 128], INT32, tag="offs")
            nc.sync.dma_start(offs[:], xids_d.ap().rearrange("a p -> p a"))
            hofs = cpool.tile([128, 2 * DK], INT32, tag="hofs")
            nc.sync.dma_start(hofs[:], hofs_d.ap().rearrange("a p -> p a"))
            maskP = cpool.tile([128, 3, 512], BF16, tag="maskP")
            nc.sync.dma_start(maskP[:], maskP_d.ap().rearrange("a g t -> g a t"))
            fcw = cpool.tile([128, DK, NCOUT], FP32, tag="fcw")
            nc.sync.dma_start(fcw[:], fcw_d.ap().rearrange("(a p) n -> p a n", p=128))
            consts = (ones_col, ones_row, cneg_row, ident, offs, hofs,
                      maskP, eps_col, ones_rows)

            # ---------------- embedding + LN (token-major) ----------------
            h = hpool.tile([128, DK, EXT], BF16, tag="h")
            for c in range(EXTP // 128):
                emb_tm = spool.tile([128, D], BF16, tag="g", bufs=12,
                                    name="emb_tm")
                nc.gpsimd.indirect_dma_start(
                    out=emb_tm[:], out_offset=None, in_=emb_d[:, :],
                    in_offset=IndirectOffsetOnAxis(ap=offs[:, c:c + 1], axis=0),
                )
                pos_sb = spool.tile([128, D], BF16, tag="g", bufs=12,
                                    name="pos_sb")
                nc.sync.dma_start(pos_sb[:], pos_d[c * 128:(c + 1) * 128, :])
                x0 = spool.tile([128, D], BF16, tag="g", bufs=12, name="x0")
                nc.vector.tensor_tensor(out=x0[:], in0=emb_tm[:], in1=pos_sb[:],
                                        op=ALU.add)
                st6 = smpool.tile([128, 2, 6], FP32, tag="st6")
                nc.vector.bn_stats(st6[:, 0, :], x0[:, 0:384])
                nc.vector.bn_stats(st6[:, 1, :], x0[:, 384:768])
                agg = smpool.tile([128, 2], FP32, tag="agg")
                nc.vector.bn_aggr(agg[:], st6[:].rearrange("p a b -> p (a b)"))
                lnv = smpool.tile([128, 1], FP32, tag="lnv")
                nc.scalar.activation(lnv[:], agg[:, 1:2], AF.Ln, bias=eps_col[:])
                rstd = smpool.tile([128, 1], FP32, tag="rstd")
                nc.scalar.activation(rstd[:], lnv[:], AF.Exp, scale=-0.5)
                hn_tm = spool.tile([128, D], BF16, tag="g", bufs=12,
                                    name="hn_tm")
                nc.vector.tensor_scalar(
                    out=hn_tm[:], in0=x0[:], scalar1=agg[:, 0:1],
                    scalar2=rstd[:], op0=ALU.subtract, op1=ALU.mult)
                # transpose to d-major
                ncols = min(128, EXT - c * 128)
                for k in range(DK):
                    ps_t = ppool.tile([128, 128], BF16, tag=f"s{k % 2}",
                                      bufs=1, name="ps_t")
                    nc.tensor.transpose(ps_t[:], hn_tm[:, k * 128:(k + 1) * 128],
                                        ident[:])
                    nc.vector.tensor_copy(
                        out=h[:, k, c * 128:c * 128 + ncols],
                        in_=ps_t[:, :ncols])

            # ---------------- layers ----------------
            # qkv for layer 0 (later layers emit qkv at the tail of the
            # previous layer so it can hide the halo collective)
            state = {"h": h}
            emit_qkv(nc, 0, state, wview, consts, pools, first=True)
            for l in range(n_layers):
                with nc.named_scope(f"L{l:02d}"):
                    layer_body(nc, l, state, wview, consts, ag_in, ag_out,
                               n_layers, pools)

            # ---------------- final mean + fc ----------------
            h = state["h"]
            hsum = smpool.tile([128, DK], FP32, tag="hsum")
            for k in range(DK):
                nc.vector.tensor_reduce(out=hsum[:, k:k + 1],
                                        in_=h[:, k, W:W + OWN],
                                        axis=AX.X, op=ALU.add)
            ps_fc = ppool.tile([NCOUT, 1], FP32, tag="s0", bufs=1, name="ps_fc")
            for k in range(DK):
                nc.tensor.matmul(ps_fc[:], fcw[:, k, :], hsum[:, k:k + 1],
                                 start=(k == 0), stop=(k == DK - 1))
            out_sb = smpool.tile([NCOUT, 1], FP32, tag="out_sb")
            nc.vector.tensor_copy(out_sb[:], ps_fc[:])
            nc.sync.dma_start(out_d[:, :], out_sb[:])

    nc.compile()
    return nc


def emit_qkv(nc, l, state, wview, consts, pools, first=False):
    """Emit Q/K/V projection matmuls for layer l reading state['h'].

    Q is emitted per 512-token chunk (caller interleaves with ln2 for l>0
    via emission order; here both chunks together for l=0). K's halo-free
    chunk goes first so the halo collective hides behind it; V's halo-
    dependent token chunks (0 and 8) go last.
    """
    hpool, bpool, spool, smpool, ppool = pools
    h = state["h"]
    scope = nc.named_scope(f"L{l:02d}_qkv")
    scope.__enter__()

    q_sb = bpool.tile([128, DK, OWN], BF16, tag="q")
    k_sb = bpool.tile([128, DK, EXT], BF16, tag="k")
    v_tm = bpool.tile([128, 9, D], BF16, tag="v")
    state["q"], state["k"], state["v"] = q_sb, k_sb, v_tm

    # V weights first: the big DMA completes during the Q/K stream
    wv_t = bpool.tile([128, DK, D], BF16, tag="wv_full")
    nc.sync.dma_start(wv_t[:], wview["wv"][l])

    wts = state.pop("qk_prefetch", None)
    if wts is None:
        # layer 0: fetch here (3-deep prefetch, alternating queues)
        wts = {}
        for i in range(2 * DK):
            nm, ko = ("wq", i) if i < DK else ("wk", i - DK)
            wt = spool.tile([128, DK, 128], BF16, tag="wt", bufs=18)
            eng = nc.sync if i % 2 == 0 else nc.scalar
            eng.dma_start(wt[:], wview[nm][l, :, :, ko * 128:(ko + 1) * 128])
            wts[i] = wt

    mm = state["mm"] = MMStream(nc, ppool)

    def q_chunk(t0, tl):
        for ko in range(DK):
            wt = wts[ko]
            ps = mm.tile()
            for ki in range(DK):
                nc.tensor.matmul(ps[:, :tl], wt[:, ki, :],
                                 h[:, ki, W + t0:W + t0 + tl],
                                 start=(ki == 0), stop=(ki == DK - 1))
            nc.vector.tensor_copy(q_sb[:, ko, t0:t0 + tl], ps[:, :tl])

    def k_chunk(t0, tl):
        for ko in range(DK):
            wt = wts[DK + ko]
            ps = mm.tile()
            for ki in range(DK):
                nc.tensor.matmul(ps[:, :tl], wt[:, ki, :],
                                 h[:, ki, t0:t0 + tl],
                                 start=(ki == 0), stop=(ki == DK - 1))
            nc.vector.tensor_copy(k_sb[:, ko, t0:t0 + tl], ps[:, :tl])

    def v_chunk(c):
        ncols = min(128, EXT - c * 128)
        for d0, dl in ((0, 512), (512, 256)):
            ps = mm.tile()
            for ki in range(DK):
                nc.tensor.matmul(ps[:ncols, :dl],
                                 h[:, ki, c * 128:c * 128 + ncols],
                                 wv_t[:, ki, d0:d0 + dl],
                                 start=(ki == 0), stop=(ki == DK - 1))
            nc.scalar.copy(v_tm[:ncols, c, d0:d0 + dl], ps[:ncols, :dl])

    # chunk order: work that needs only ln2-c0 of the previous layer comes
    # first (Q-c0, V tokens 128..512) so the producing layer's ln2-c1 scalar
    # chain hides behind it; halo-dependent pieces (K cols 0:512 + tail,
    # V chunks 0 and 8) go last to hide the collective.
    q_chunk(0, 512)
    v_chunk(1)
    q_chunk(512, 512)
    for (t0, tl) in TC_EXT:
        k_chunk(t0, tl)
    for c in (2, 3, 4, 5, 6, 7, 0, 8):
        v_chunk(c)
    scope.__exit__(None, None, None)


class MMStream:
    """Ping-pong PSUM tile dispenser for the big-GEMM streams (banks s0/s1)."""

    def __init__(self, nc, ppool):
        self.nc, self.ppool, self.i = nc, ppool, 0

    def tile(self, name="ps_mm"):
        t = self.ppool.tile([128, 512], FP32, tag=f"s{self.i % 2}", bufs=1,
                            name=name)
        self.i += 1
        return t


def ln_d_major(nc, buf, off, t0, tl, consts, spool, smpool, ppool):
    """In-place layernorm over d for one <=512-token chunk of d-major bf16
    activations at buf[:, k, off+t0 : off+t0+tl].

    Stats land in a single PSUM bank (sum at partition 0, sumsq at partition
    32 via tile_position) and are copied out immediately so the bank frees
    fast; the scalar chain then runs off-PSUM; the row broadcasts use the
    stream banks s0/s1.
    """
    ones_col, ones_row, cneg_row = consts[0], consts[1], consts[2]
    eps_row = consts[7][0:1, :]
    a0 = off + t0

    st = ppool.tile([33, 512], FP32, tag="lnst", bufs=1, name="ln_st")
    for k in range(6):
        sqt = spool.tile([128, 512], BF16, tag="sqt", bufs=2)
        nc.scalar.square(sqt[:, :tl], buf[:, k, a0:a0 + tl])
        nc.tensor.matmul(st[0:1, :tl], ones_col[:], buf[:, k, a0:a0 + tl],
                         start=(k == 0), stop=(k == 5), skip_group_check=True)
        nc.tensor.matmul(st[32:33, :tl], ones_col[:], sqt[:, :tl],
                         start=(k == 0), stop=(k == 5),
                         tile_position=(0, 32), skip_group_check=True)

    def row(nm, tag="lnrow", bufs=2):
        return smpool.tile([1, 512], FP32, tag=tag, bufs=bufs, name=nm)

    # scalar chain reads the stats straight from PSUM (st row 0 = sum,
    # row 32 = sumsq); the row broadcast runs on GpSimd so the PE stream
    # banks are never touched by layernorm.
    sum_sb = row("sum_sb", tag="ln_sum", bufs=2)
    nc.vector.tensor_copy(sum_sb[:, :tl], st[0:1, :tl])
    t1 = row("t1")
    nc.vector.tensor_tensor(out=t1[:, :tl], in0=sum_sb[:, :tl],
                            in1=sum_sb[:, :tl], op=ALU.mult)
    t2 = row("t2")
    nc.vector.tensor_scalar(out=t2[:, :tl], in0=t1[:, :tl],
                            scalar1=-1.0 / D, scalar2=None, op0=ALU.mult)
    diff = row("diff")
    nc.vector.tensor_tensor(out=diff[:, :tl], in0=st[32:33, :tl],
                            in1=t2[:, :tl], op=ALU.add)
    dpos = row("dpos")
    nc.vector.tensor_scalar(out=dpos[:, :tl], in0=diff[:, :tl],
                            scalar1=0.0, scalar2=None, op0=ALU.max)
    lnv = row("lnv")
    nc.scalar.activation(lnv[:, :tl], dpos[:, :tl], AF.Ln,
                         bias=eps_row, scale=1.0 / D)
    rstd = row("rstd")
    nc.scalar.activation(rstd[:, :tl], lnv[:, :tl], AF.Exp, scale=-0.5)
    nmr = row("nmr")
    nc.vector.tensor_scalar(out=nmr[:, :tl], in0=sum_sb[:, :tl],
                            scalar1=-1.0 / D, scalar2=None, op0=ALU.mult)
    both_bf = smpool.tile([1, 2, 512], BF16, tag="lnrow_bf", bufs=2,
                          name="both_bf")
    nc.vector.tensor_copy(both_bf[:, 0, :tl], rstd[:, :tl])
    nc.vector.tensor_tensor(out=both_bf[:, 1, :tl], in0=nmr[:, :tl],
                            in1=rstd[:, :tl], op=ALU.mult)
    bc = spool.tile([128, 2, 512], BF16, tag="rbs", bufs=2, name="bc")
    nc.gpsimd.partition_broadcast(bc[:, :, :tl].rearrange("p a t -> p (a t)"),
                                  both_bf[:, :, :tl].rearrange("p a t -> p (a t)"))
    for k in range(DK):
        tmp = spool.tile([128, 512], BF16, tag="lnap", bufs=2)
        nc.vector.tensor_tensor(out=tmp[:, :tl], in0=buf[:, k, a0:a0 + tl],
                                in1=bc[:, 0, :tl], op=ALU.mult)
        nc.vector.tensor_tensor(out=buf[:, k, a0:a0 + tl],
                                in0=tmp[:, :tl], in1=bc[:, 1, :tl],
                                op=ALU.add)


def layer_body(nc, l, state, wview, consts, ag_in, ag_out, n_layers, pools):
    hpool, bpool, spool, smpool, ppool = pools
    (ones_col, ones_row, cneg_row, ident, offs, hofs, maskP,
     eps_col, ones_rows) = consts
    h = state["h"]
    q_sb, k_sb, v_tm = state["q"], state["k"], state["v"]
    mm = state["mm"]

    # Wo weights fetched now so they are resident long before the Wo matmuls
    wo_tiles = []
    for i in range(DK):
        wt = spool.tile([128, DK, 128], BF16, tag="wt", bufs=18)
        eng = nc.sync if i % 2 == 0 else nc.scalar
        eng.dma_start(wt[:], wview["wo"][l, :, :, i * 128:(i + 1) * 128])
        wo_tiles.append(wt)

    # ---------------- attention ----------------
    # Per query chunk qc and head-pair j: 4 score matmuls -> exp (ACT) ->
    # 0/1 mask multiply (GpSimd) -> column-sum denominators into one shared
    # PSUM bank (tile_position slots) + PV matmuls -> one batched reciprocal
    # per qc -> K=1 broadcast matmuls (reusing the score banks) -> normalize
    # multiplies (DVE). The big-GEMM streams (Wo etc.) weave into the stalls.
    a_sb = bpool.tile([128, DK, OWN], BF16, tag="attn")
    attn_scope = nc.named_scope(f"L{l:02d}_attn")
    attn_scope.__enter__()
    for qc in range(NQC):
        si = 0 if qc == 0 else (2 if qc == NQC - 1 else 1)
        g0 = qc * 128
        seb = ppool.tile([97, 512], FP32, tag="seb", bufs=1, name="seb")
        pvsb = []
        for j in range(H // 2):
            sc = ppool.tile([128, 512], FP32, tag=f"sc{j % 2}", bufs=1,
                            name="sc")
            for ph, ro in ((0, 0), (1, 64)):
                q_ap = q_sb[ro:ro + 64, j, g0:g0 + 128]
                nc.tensor.matmul(sc[:, 128 * ph:128 * ph + 128],
                                 k_sb[ro:ro + 64, j, g0:g0 + 128], q_ap,
                                 start=True, stop=True, skip_group_check=True)
                nc.tensor.matmul(sc[:64, 256 + 128 * ph:384 + 128 * ph],
                                 k_sb[ro:ro + 64, j, g0 + 128:g0 + GW], q_ap,
                                 start=True, stop=True, skip_group_check=True)
            eA = spool.tile([128, 256], BF16, tag="eA", bufs=4, name="eA")
            nc.scalar.activation(eA[:], sc[:, 0:256], AF.Exp)
            eB = spool.tile([64, 256], BF16, tag="eB", bufs=4, name="eB")
            nc.scalar.activation(eB[:], sc[:64, 256:512], AF.Exp)
            eAm = spool.tile([128, 256], BF16, tag="eAm", bufs=4, name="eAm")
            nc.vector.tensor_tensor(out=eAm[:], in0=eA[:],
                                    in1=maskP[:, si, 0:256], op=ALU.mult)
            eBm = spool.tile([64, 256], BF16, tag="eBm", bufs=4, name="eBm")
            nc.vector.tensor_tensor(out=eBm[:], in0=eB[:],
                                    in1=maskP[:64, si, 256:512], op=ALU.mult)
            s, cc = divmod(j, 2)
            nc.tensor.matmul(seb[32 * s:32 * s + 1, 256 * cc:256 * cc + 256],
                             ones_col[:, :], eAm[:], start=True, stop=False,
                             tile_position=(0, 32 * s), skip_group_check=True)
            nc.tensor.matmul(seb[32 * s:32 * s + 1, 256 * cc:256 * cc + 256],
                             ones_col[0:64, :], eBm[:], start=False, stop=True,
                             tile_position=(0, 32 * s), skip_group_check=True)
            pvp = ppool.tile([128, 512], FP32, tag=f"pv{j % 2}", bufs=1,
                             name="pvp")
            for ph, po in ((0, 0), (1, 64)):
                pv = pvp[po:po + 64, 128 * ph:128 * ph + 128]
                nc.tensor.matmul(
                    pv, v_tm[:, qc, 128 * j + 64 * ph:128 * j + 64 * ph + 64],
                    eAm[:, 128 * ph:128 * ph + 128], start=True, stop=False,
                    tile_position=(0, po), skip_group_check=True)
                nc.tensor.matmul(
                    pv, v_tm[:64, qc + 1, 128 * j + 64 * ph:128 * j + 64 * ph + 64],
                    eBm[:, 128 * ph:128 * ph + 128], start=False, stop=True,
                    tile_position=(0, po), skip_group_check=True)
            pv_sb = spool.tile([128, 256], BF16, tag="pv_sb", bufs=8,
                               name="pv_sb")
            nc.scalar.copy(pv_sb[:], pvp[:, 0:256])
            pvsb.append(pv_sb)
        rc_f = spool.tile([97, 512], FP32, tag="rc_f", bufs=1, name="rc_f")
        nc.vector.reciprocal_approx_fast(out=rc_f[:], in_=seb[0:97, 0:512])
        rc = spool.tile([97, 512], BF16, tag="rc", bufs=2, name="rc")
        nc.vector.tensor_copy(rc[:], rc_f[:])
        for s in range(3):
            rbq = ppool.tile([128, 512], FP32, tag="seb", bufs=1,
                             name="rbq")
            nc.tensor.matmul(rbq[:], ones_rows[32 * s:32 * s + 1, :],
                             rc[32 * s:32 * s + 1, 0:512],
                             start=True, stop=True, tile_position=(32 * s, 0),
                             skip_group_check=True)
            for cc in range(2):
                j = 2 * s + cc
                for ph, po in ((0, 0), (1, 64)):
                    nc.vector.tensor_tensor(
                        out=a_sb[po:po + 64, j, g0:g0 + 128],
                        in0=pvsb[j][po:po + 64, 128 * ph:128 * ph + 128],
                        in1=rbq[po:po + 64, 256 * cc + 128 * ph:
                                256 * cc + 128 * ph + 128],
                        op=ALU.mult)
    attn_scope.__exit__(None, None, None)

    # ---------------- Wo + residual -> LN1 (in-place) -> h2 ----------------
    wo_scope = nc.named_scope(f"L{l:02d}_wo_ln1")
    wo_scope.__enter__()
    h2 = bpool.tile([128, DK, OWN], BF16, tag="h2")
    for (t0, tl) in TC_OWN:
        for ko in range(DK):
            wt = wo_tiles[ko]
            ps = mm.tile("ps_wo")
            for ki in range(DK):
                nc.tensor.matmul(ps[:, :tl], wt[:, ki, :], a_sb[:, ki, t0:t0 + tl],
                                 start=(ki == 0), stop=(ki == DK - 1))
            nc.vector.tensor_tensor(out=h2[:, ko, t0:t0 + tl],
                                    in0=h[:, ko, W + t0:W + t0 + tl],
                                    in1=ps[:, :tl], op=ALU.add)
        ln_d_major(nc, h2, 0, t0, tl, consts, spool, smpool, ppool)
    wo_scope.__exit__(None, None, None)

    # ---------------- FFN -> residual -> LN2 (in-place) -> h3 --------------
    ffn_scope = nc.named_scope(f"L{l:02d}_ffn")
    ffn_scope.__enter__()
    h3 = hpool.tile([128, DK, EXT], BF16, tag="h")
    for ci, (t0, tl) in enumerate(TC_OWN):
        # W1 phase: stream banks; gelu outputs parked in SBUF pair-tiles
        w1pre = {}

        def w1_fetch(j):
            if j < DFFK:
                w1t = spool.tile([128, DK, 128], BF16, tag="w1t", bufs=4)
                eng = nc.sync if j % 2 == 0 else nc.scalar
                eng.dma_start(w1t[:],
                              wview["w1"][l, :, :, j * 128:(j + 1) * 128])
                w1pre[j] = w1t
        for jf in range(3):
            w1_fetch(jf)
        gt = [spool.tile([128, 2, 512], BF16, tag="g", bufs=12,
                         name=f"g{jp}") for jp in range(DFFK // 2)]
        w2pre = {}

        def w2_fetch(ko):
            if ko < DK:
                w2t = spool.tile([128, DFFK, 128], BF16, tag="w2t", bufs=2)
                eng = nc.sync if ko % 2 == 0 else nc.scalar
                eng.dma_start(w2t[:],
                              wview["w2"][l, :, :, ko * 128:(ko + 1) * 128])
                w2pre[ko] = w2t
        w2_fetch(0)
        w2_fetch(1)
        for j in range(DFFK):
            w1_fetch(j + 3)
            w1t = w1pre.pop(j)
            ps1 = mm.tile("ps1")
            for ki in range(DK):
                nc.tensor.matmul(ps1[:, :tl], w1t[:, ki, :],
                                 h2[:, ki, t0:t0 + tl],
                                 start=(ki == 0), stop=(ki == DK - 1))
            nc.scalar.activation(gt[j // 2][:, j % 2, :tl], ps1[:, :tl],
                                 AF.Gelu)
        # W2 phase: ko-major over resident gelu tiles, 2 cycling acc banks
        for ko in range(DK):
            w2_fetch(ko + 2)
            w2t = w2pre.pop(ko)
            acc = ppool.tile([128, 512], FP32, tag=f"pv{ko % 2}", bufs=1,
                             name="acc")
            for j in range(DFFK):
                nc.tensor.matmul(acc[:, :tl], w2t[:, j, :],
                                 gt[j // 2][:, j % 2, :tl],
                                 start=(j == 0), stop=(j == DFFK - 1))
            nc.vector.tensor_tensor(out=h3[:, ko, W + t0:W + t0 + tl],
                                    in0=h2[:, ko, t0:t0 + tl],
                                    in1=acc[:, :tl], op=ALU.add)
        if ci == 0 and l < n_layers - 1:
            # prefetch next layer's Q/K weights during FFN
            nxt = {}
            for i in range(2 * DK):
                nm, ko = ("wq", i) if i < DK else ("wk", i - DK)
                wt = spool.tile([128, DK, 128], BF16, tag="wt", bufs=18)
                eng = nc.sync if i % 2 == 0 else nc.scalar
                eng.dma_start(wt[:],
                              wview[nm][l + 1, :, :, ko * 128:(ko + 1) * 128])
                nxt[i] = wt
            state["qk_prefetch"] = nxt
        # ln2 for this chunk right away: unlocks next-layer Q chunk 0 early
        ln_d_major(nc, h3, W, t0, tl, consts, spool, smpool, ppool)
    ffn_scope.__exit__(None, None, None)

    state["h"] = h3
    # ---------------- halo exchange + next-layer qkv ----------------
    if l < n_layers - 1:
        with nc.named_scope(f"L{l:02d}_halo"):
            agi = ag_in[l].ap().rearrange("(a p) s c -> p a s c", p=128)
            nc.gpsimd.dma_start(agi[:, :, 0, :], h3[:, :, W:2 * W])
            nc.gpsimd.dma_start(agi[:, :, 1, :], h3[:, :, OWN:W + OWN])
            nc.gpsimd.collective_compute(
                "AllGather", ALU.bypass,
                replica_groups=[list(range(NC_CORES))],
                ins=[ag_in[l].ap()], outs=[ag_out[l].ap()],
            )
            agv = ag_out[l].ap().rearrange("s d b c -> (s d b) c")
            for k in range(DK):
                nc.gpsimd.indirect_dma_start(
                    out=h3[:, k, 0:W], out_offset=None, in_=agv[:, :],
                    in_offset=IndirectOffsetOnAxis(ap=hofs[:, k:k + 1], axis=0),
                )
                nc.gpsimd.indirect_dma_start(
                    out=h3[:, k, W + OWN:EXT], out_offset=None, in_=agv[:, :],
                    in_offset=IndirectOffsetOnAxis(
                        ap=hofs[:, DK + k:DK + k + 1], axis=0),
                )
        emit_qkv(nc, l + 1, state, wview, consts, pools)


_NC_CACHE = {}


def _get_nc(n_layers):
    if n_layers not in _NC_CACHE:
        _NC_CACHE[n_layers] = build_nc(n_layers)
    return _NC_CACHE[n_layers]


def make_in_maps(x, emb, pos_emb, tok_emb, Wq, Wk, Wv, Wo, W1, W2, fc_w,
                 n_layers):
    x = np.asarray(x)
    bf = lambda a: np.ascontiguousarray(np.asarray(a), dtype=np.float32).astype(
        ml_dtypes.bfloat16)
    scale = 1.0 / np.sqrt(np.float32(DH))
    shared = {
        "emb": bf(emb),
        "wq": bf(np.asarray(Wq)[:n_layers] * scale),
        "wk": bf(np.asarray(Wk)[:n_layers]),
        "wv": bf(np.asarray(Wv)[:n_layers]),
        "wo": bf(np.asarray(Wo)[:n_layers]),
        "w1": bf(np.asarray(W1)[:n_layers]),
        "w2": bf(np.asarray(W2)[:n_layers]),
        "fcw": np.ascontiguousarray(np.asarray(fc_w), dtype=np.float32),
        "ident": np.eye(128, dtype=ml_dtypes.bfloat16),
    }
    postok_full = (np.asarray(pos_emb)[1:S + 1] + np.asarray(tok_emb)[0]
                   ).astype(np.float32)

    in_maps = []
    for c in range(NC_CORES):
        b, s_idx = divmod(c, SHARDS_PER_B)
        own0 = s_idx * OWN
        ext_pos = np.arange(own0 - W, own0 - W + EXTP)
        valid = (ext_pos >= 0) & (ext_pos < S)
        pos_c = np.clip(ext_pos, 0, S - 1)
        xids = np.where(valid, x[b][pos_c], 1).astype(np.int32)
        postok = np.where(valid[:, None], postok_full[pos_c], 0.0
                          ).astype(ml_dtypes.bfloat16)
        # 0/1 masks: scores[g, t] for query chunk qc; key global position is
        # own0 - W + qc*128 + g, query global position own0 + qc*128 + t.
        gi = np.arange(GW)[:, None]
        ti = np.arange(128)[None, :]
        band = np.abs((gi - W) - ti) <= W
        mP = np.zeros((3, 128, 512), ml_dtypes.bfloat16)
        for si, qc in ((0, 0), (1, 1), (2, NQC - 1)):
            kpos = own0 - W + qc * 128 + np.arange(GW)
            ok = band & ((kpos >= 0) & (kpos < S))[:, None]
            m = np.where(ok, 1.0, 0.0).astype(ml_dtypes.bfloat16)
            mP[si, :, 0:128] = m[:128]
            mP[si, :, 128:256] = m[:128]
            mP[si, :64, 256:384] = m[128:]
            mP[si, :64, 384:512] = m[128:]
        # halo gather offsets into ag_out viewed as rows [(8*768*2), 32]
        slot_l, slot_r = max(c - 1, 0), min(c + 1, NC_CORES - 1)
        p = np.arange(128)
        hofs = np.empty((2 * DK, 128), np.int32)
        for k in range(DK):
            hofs[k] = (slot_l * D + k * 128 + p) * 2 + 1
            hofs[DK + k] = (slot_r * D + k * 128 + p) * 2 + 0
        in_maps.append({
            **shared, "xids": xids.reshape(EXTP // 128, 128),
            "postok": postok, "maskP": mP, "hofs": hofs,
        })
    return in_maps


def kernel(x, emb, pos_emb, tok_emb, emb_ln_s, emb_ln_b, Wq, bq, Wk, bk,
           Wv, bv, Wo, bo, ln1_s, ln1_b, W1, b1, W2, b2, ln2_s, ln2_b,
           fc_w, fc_b, _n_layers=None, _results_hook=None):
    n_layers = _n_layers if _n_layers is not None else L
    for z in (bq, bk, bv, bo, b1, b2, emb_ln_b, ln1_b, ln2_b):
        assert not np.any(np.asarray(z)), "nonzero biases unsupported"
    for o in (emb_ln_s, ln1_s, ln2_s):
        assert np.all(np.asarray(o) == 1.0), "non-unit LN scales unsupported"

    in_maps = make_in_maps(x, emb, pos_emb, tok_emb, Wq, Wk, Wv, Wo, W1, W2,
                           fc_w, n_layers)
    nc = _get_nc(n_layers)
    res = run_bass_kernel_spmd(nc, in_maps, list(range(NC_CORES)))
    if _results_hook is not None:
        _results_hook(res)
    out = np.zeros((B, NCOUT), np.float32)
    for c in range(NC_CORES):
        out[c // SHARDS_PER_B] += res.results[c]["out"][:, 0]
    out = out / np.float32(S) + np.asarray(fc_b, np.float32)
    return out
